# revision 50
# baseline (speedup 1.0000x reference)
"""nn_BayesianLayer — reparameterized Bayesian linear layer + inverted dropout
on 8 TRN2 NeuronCores (data-parallel over the 65536-row batch).

reference:
  w = w_mu + softplus(w_rho) * w_eps            [512, 512]
  b = b_mu + softplus(b_rho) * b_eps            [512]
  y = (x @ w.T + b) * (drop_u >= 0.2) / 0.8     [65536, 512]

Sharding: x and drop_u split into 8 row-shards of 8192; weights replicated.
Each core runs the same single-core Bass/Tile graph (SPMD, no collectives);
outputs are gathered on the host.

This is a memory-regime problem (~51MB/core of fp32 traffic at a 358GB/s
per-core HBM ceiling), so the kernel is built around shrinking bytes moved
(the rel-err budget is 2e-2; measured end-to-end rel err ~2e-3):
 - x is fed host-transposed AND bf16 (xT [512, 8192], 8MB vs 16MB fp32).
 - drop_u enters as its information content: a host-encoded keep mask with
   fp8(e4m3) bit patterns {0.0, 1.0} in a uint8 tensor, transposed to
   [512, 8192] (4MB vs 16MB). On device it is bitcast to float8e4.
 - y leaves the device as bf16 (8MB) and is upcast to fp32 on the host.
 - the weight/rho/eps tensors are fed host-transposed bf16 (1.5MB).
Total ~21.5MB/core -> ~60us DMA roofline (vs ~142us for the fp32 layout).

The matmul is emitted output-transposed: yT[n, m] = sum_k w'T[k, n]*xT[k, m].
The stationary operand is a [128, 128] chunk of w'T (reused across 4 moving
passes - 4x fewer PE weight loads), and the bias lands on the PSUM partition
axis, so it can ride the per-partition "scalar" operand of a single fused
DVE/GPSIMD op or the ACT bias port -- no 5th bias matmul pass (TensorE stays
at its 4-pass compute floor, ~55us at 2.4GHz). Per 128x512 psum tile ONE
fused op applies bias+mask+downcast:
    yT_tile = (psum + b'[n]) * mask      (scalar_tensor_tensor, add/mult)
Tiles alternate between the DVE (direct from PSUM) and an ACT(bias via
Identity-activation bias port, PSUM->SBUF bf16) -> GPSIMD(mask multiply)
pipeline, because GPSIMD has no PSUM port and the DVE alone (1x mode from
PSUM + drains) would be the bottleneck.

Weight prologue (on device): w' = 1.25*(w_mu + softplus(w_rho)*w_eps).T in
bf16. softplus(rho) = ln1p(exp(rho)) for rho<0 (always true here); exp on
ACT, ln1p via a 4-term poly on DVE/GPSIMD (chunks split across both so the
serial chain that gates the first matmul stays short; ACT tables lack
Softplus/Ln). The 1/(1-p) dropout scale is folded into w', b'.

Main loop: 4 groups of 2048 batch rows; per group 2MB xs + 1MB mask slabs in,
2MB yT out, each split half/half across the SP and ACT HWDGE rings so both
descriptor rings stay busy at every instant.
"""

import numpy as np
import ml_dtypes

import concourse.bass as bass
import concourse.mybir as mybir
from concourse import bacc, tile
from concourse.bass import ts
from concourse.bass_utils import run_bass_kernel_spmd

AF = mybir.ActivationFunctionType
ALU = mybir.AluOpType

N_CORES = 8
B, IN, OUT = 65536, 512, 512
BS = B // N_CORES          # 8192 rows per core
P = 128
KC = IN // P               # 4 contraction chunks
SC = OUT // P              # 4 output (n) slices
GROUPS = 4                 # batch groups per core (default)
TW = 512                   # moving-tile width (psum free dim; max legal 512)
CW = 512                   # consumer-op chunk width
DROP = 0.2
SCALE = 1.0 / (1.0 - DROP)
FP8_ONE = 0x38             # float8e4 bit pattern of 1.0

# softplus(rho) on rho in [-3.7, -2.3] (rho = -3.0 + 0.1*randn; +-7 sigma)
# as a cubic Chebyshev fit in nested (u + b)*rho form -- 3 DVE ops, no Exp,
# no ACT on the prologue critical path. Max abs err 7.8e-5 (vs the ~4e-3
# bf16 rounding floor of this kernel). SP_C3 carries the folded 1/(1-p)
# dropout scale.
SP_B2 = 12.357892265882713
SP_B1 = 54.021235707244024
SP_B0 = 84.88210397568128
SP_C3 = 0.006898174577132781


def build_kernel(reps=None, dve8=6,
                 groups=(1024, 2048, 2048, 2048, 1024),
                 tw=TW, xbufs=3, obufs=2, out_split=True, order=2,
                 dma_only=False, pe_only=False):
    if isinstance(groups, int):
        groups = (BS // groups,) * groups
    assert sum(groups) == BS
    nc = bacc.Bacc(None, target_bir_lowering=False, debug=False)
    f32 = mybir.dt.float32
    bf16 = mybir.dt.bfloat16
    u8 = mybir.dt.uint8
    f8 = mybir.dt.float8e4

    xt = nc.declare_dram_parameter("xt", [IN, BS], bf16, isOutput=False)
    wmu = nc.declare_dram_parameter("wmu", [IN, OUT], bf16, isOutput=False)
    wrho = nc.declare_dram_parameter("wrho", [IN, OUT], bf16, isOutput=False)
    weps = nc.declare_dram_parameter("weps", [IN, OUT], bf16, isOutput=False)
    ball = nc.declare_dram_parameter("ball", [P, 3, SC], f32, isOutput=False)
    mk = nc.declare_dram_parameter("mk", [OUT, BS], u8, isOutput=False)
    yo = nc.declare_dram_parameter("yo", [OUT, BS], bf16, isOutput=True)

    xt_r = xt[:, :].rearrange("(k p) m -> p k m", p=P)    # [128, KC, BS]
    wmu_r = wmu[:, :].rearrange("(k p) n -> p k n", p=P)  # [128, KC, OUT]
    wrho_r = wrho[:, :].rearrange("(k p) n -> p k n", p=P)
    weps_r = weps[:, :].rearrange("(k p) n -> p k n", p=P)
    mk_r = mk[:, :].rearrange("(s p) m -> p s m", p=P)    # [128, SC, BS]
    yo_r = yo[:, :].rearrange("(s p) m -> p s m", p=P)

    with tile.TileContext(nc) as tc:
        with (
            tc.tile_pool(name="wt", bufs=1) as wt_pool,
            tc.tile_pool(name="prol", bufs=2) as prol_pool,
            tc.tile_pool(name="bias", bufs=1) as bias_pool,
            tc.tile_pool(name="xs", bufs=xbufs) as x_pool,
            tc.tile_pool(name="mks", bufs=xbufs) as mk_pool,
            tc.tile_pool(name="outs", bufs=obufs) as out_pool,
            tc.tile_pool(name="tmp", bufs=6) as tmp_pool,
            tc.tile_pool(name="ps", bufs=8 * 512 // tw,
                         space="PSUM") as psum_pool,
        ):
            # ---- weight prologue: w'T = 1.25*(mu + ln1p(exp(rho))*eps).T,
            # per k-chunk in bf16: exp on ACT; a = (t*-0.5 + 1) on DVE; then
            # b = t*eps, c = a*b, w = c+mu as tensor-tensor ops alternating
            # DVE/GPSIMD per chunk; cast+scale on ACT. ----
            # mu arrives host-prescaled by 1.25 (the dropout scale); the
            # same 1.25 is folded into SP_C3, so w' = mu' + sp'(rho)*eps
            # needs no extra scale/cast: the final add writes the bf16 wt
            # tile directly. Poly on DVE; tails alternate DVE/GPSIMD.
            # DMA order is tuned for the serial start: rho first (gates the
            # poly), then group 0's input slabs, then eps/mu/bias.
            from collections import Counter
            size_count = Counter(groups)

            def prefetch_group(m0, gb):
                h = gb // 2
                bfs = min(size_count[gb], xbufs)
                xs = x_pool.tile([P, KC, gb], bf16, tag=f"xs{gb}",
                                 name=f"xs{gb}", bufs=bfs)
                nc.sync.dma_start(out=xs[:, :, :h],
                                  in_=xt_r[:, :, m0:m0 + h])
                nc.scalar.dma_start(out=xs[:, :, h:],
                                    in_=xt_r[:, :, m0 + h:m0 + gb])
                mks = mk_pool.tile([P, SC, gb], u8, tag=f"mks{gb}",
                                   name=f"mks{gb}", bufs=bfs)
                nc.sync.dma_start(out=mks[:, :, :h],
                                  in_=mk_r[:, :, m0:m0 + h])
                nc.scalar.dma_start(out=mks[:, :, h:],
                                    in_=mk_r[:, :, m0 + h:m0 + gb])
                return xs, mks

            rho_a = prol_pool.tile([P, KC, OUT], bf16, tag="rho", bufs=1)
            mu_a = prol_pool.tile([P, KC, OUT], bf16, tag="mu", bufs=1)
            eps_a = prol_pool.tile([P, KC, OUT], bf16, tag="eps", bufs=1)
            g0_tiles = None
            if order == 0:
                nc.sync.dma_start(out=rho_a[:], in_=wrho_r[:, :])
                if reps is None:
                    g0_tiles = prefetch_group(0, groups[0])
                nc.scalar.dma_start(out=eps_a[:], in_=weps_r[:, :])
                nc.scalar.dma_start(out=mu_a[:], in_=wmu_r[:, :])
            elif order == 1:
                nc.scalar.dma_start(out=rho_a[:], in_=wrho_r[:, :])
                nc.sync.dma_start(out=eps_a[:], in_=weps_r[:, :])
                nc.sync.dma_start(out=mu_a[:], in_=wmu_r[:, :])
                if reps is None:
                    g0_tiles = prefetch_group(0, groups[0])
            else:
                nc.sync.dma_start(out=rho_a[:], in_=wrho_r[:, :])
                nc.scalar.dma_start(out=eps_a[:], in_=weps_r[:, :])
                nc.sync.dma_start(out=mu_a[:], in_=wmu_r[:, :])
                if reps is None:
                    g0_tiles = prefetch_group(0, groups[0])
            wt = []
            for k in range(KC):
                rho_k = rho_a[:, k]
                sp = prol_pool.tile([P, OUT], f32, tag="sp")
                nc.vector.scalar_tensor_tensor(
                    sp[:], rho_k, SP_B2, rho_k, ALU.add, ALU.mult)
                nc.vector.scalar_tensor_tensor(
                    sp[:], sp[:], SP_B1, rho_k, ALU.add, ALU.mult)
                nc.vector.tensor_scalar(sp[:], sp[:], SP_B0, SP_C3 * SCALE,
                                        ALU.add, ALU.mult)
                eng = nc.gpsimd if k % 2 == 0 else nc.vector
                eng.tensor_mul(sp[:], sp[:], eps_a[:, k])
                wtk = wt_pool.tile([P, OUT], bf16, tag=f"wt{k}")
                eng.tensor_add(wtk[:], sp[:], mu_a[:, k])
                wt.append(wtk)

            # ---- bias prologue: b' as a [128, SC] per-partition table;
            # column s is the bias vector for output slice s. The three
            # bias tensors arrive packed in one [P, 3, SC] DMA. ----
            ball_t = bias_pool.tile([P, 3, SC], f32, tag="ball")
            nc.scalar.dma_start(out=ball_t[:], in_=ball[:, :, :])
            bmu_t, brho_t, beps_t = ball_t[:, 0], ball_t[:, 1], ball_t[:, 2]
            spb = bias_pool.tile([P, SC], f32, tag="spb")
            nc.vector.scalar_tensor_tensor(
                spb[:], brho_t, SP_B2, brho_t, ALU.add, ALU.mult)
            nc.vector.scalar_tensor_tensor(
                spb[:], spb[:], SP_B1, brho_t, ALU.add, ALU.mult)
            nc.vector.tensor_scalar(spb[:], spb[:], SP_B0, SP_C3 * SCALE,
                                    ALU.add, ALU.mult)
            nc.vector.tensor_mul(spb[:], spb[:], beps_t)
            b_vec = bias_pool.tile([P, SC], f32, tag="bvec")
            nc.vector.tensor_add(b_vec[:], spb[:], bmu_t)

            # ---- main loop (uneven groups: small first group so output
            # stores start early, small last group so the tail drains) ----
            c_counter = [0]
            N_CHUNKS = SC * BS // CW     # 64 consumer chunks total

            def emit_group(m0, gb, pre=None):
                h = gb // 2
                if pe_only:
                    bfs = min(size_count[gb], xbufs)
                    xs = x_pool.tile([P, KC, gb], bf16, tag=f"xs{gb}",
                                     name=f"xs{gb}", bufs=bfs)
                    nc.sync.dma_start(out=xs[:, :, :h],
                                      in_=xt_r[:, :, m0:m0 + h])
                    nc.scalar.dma_start(out=xs[:, :, h:],
                                        in_=xt_r[:, :, m0 + h:m0 + gb])
                    for s in range(SC):
                        pss = [psum_pool.tile([P, tw], f32, tag="ps",
                                              name=f"pso{mb}")
                               for mb in range(gb // tw)]
                        for k in range(KC):
                            for mb in range(gb // tw):
                                nc.tensor.matmul(
                                    pss[mb][:], wt[k][:, ts(s, P)],
                                    xs[:, k, mb * tw:(mb + 1) * tw],
                                    start=(k == 0), stop=(k == KC - 1))
                    return
                xs, mks = pre if pre is not None else prefetch_group(m0, gb)
                outs = out_pool.tile([P, SC, gb], bf16, tag=f"outs{gb}",
                                     name=f"outs{gb}",
                                     bufs=min(size_count[gb], obufs))
                mbn = gb // tw if not dma_only else 0
                for s in range(SC):
                    # k-outer / mb-inner: consecutive matmuls share the
                    # stationary wt[k][:, s], so dedup_ldweights() can strip
                    # the redundant PE weight reloads (mbn-1 of every mbn).
                    pss = [psum_pool.tile([P, tw], f32, tag="ps",
                                          name=f"ps{mb}")
                           for mb in range(mbn)]
                    for k in range(KC):
                        for mb in range(mbn):
                            nc.tensor.matmul(
                                pss[mb][:], wt[k][:, ts(s, P)],
                                xs[:, k, mb * tw:(mb + 1) * tw],
                                start=(k == 0), stop=(k == KC - 1))
                    for mb in range(mbn):
                        ps = pss[mb]
                        # consumers in CW-wide chunks split across engines:
                        # A: one fused DVE op straight from PSUM;
                        # B: ACT bias-add (psum->sbuf bf16) + GPSIMD mask.
                        for q in range(tw // CW):
                            c_idx = c_counter[0]
                            c_counter[0] += 1
                            c0 = mb * tw + q * CW
                            o_sl = outs[:, s, c0:c0 + CW]
                            m_sl = mks[:, s, c0:c0 + CW].bitcast(f8)
                            p_sl = ps[:, ts(q, CW)]
                            if c_idx % 8 < dve8 or c_idx >= N_CHUNKS - 4:
                                nc.vector.scalar_tensor_tensor(
                                    o_sl, p_sl, b_vec[:, ts(s, 1)], m_sl,
                                    ALU.add, ALU.mult)
                            else:
                                tmp = tmp_pool.tile([P, CW], bf16, tag="tmp")
                                nc.scalar.activation(
                                    tmp[:], p_sl, AF.Identity,
                                    bias=b_vec[:, ts(s, 1)], scale=1.0)
                                nc.gpsimd.tensor_mul(o_sl, tmp[:], m_sl)
                    if out_split:
                        # store each n-slice as soon as its consumers finish
                        eng = nc.scalar if s % 2 == 0 else nc.sync
                        src = outs[:, s] if not dma_only else xs[:, s]
                        eng.dma_start(out=yo_r[:, s, m0:m0 + gb], in_=src)
                if not out_split:
                    nc.scalar.dma_start(out=yo_r[:, :, m0:m0 + h],
                                        in_=outs[:, :, :h])
                    nc.sync.dma_start(out=yo_r[:, :, m0 + h:m0 + gb],
                                      in_=outs[:, :, h:])

            def emit_all(first_pre):
                m0 = 0
                for gi, gb in enumerate(groups):
                    emit_group(m0, gb, pre=first_pre if gi == 0 else None)
                    m0 += gb

            if reps is None:
                emit_all(g0_tiles)
            else:
                with tc.For_i(0, reps) as _:
                    emit_all(None)

    dedup_ldweights(nc)
    nc.finalize()
    return nc


def dedup_ldweights(nc):
    """Remove PE weight reloads that re-load the stationary already resident
    in the PE array. The tile scheduler emits one InstLdweights per matmul;
    a Matmult NOT preceded by its own Ldweights uses the resident stationary
    (HW-verified: exact results, transitions included). Each removed load
    saves ~128 PE cycles. Dependency edges of a removed Ldweights are merged
    into the next Matmult so tile-level sync is preserved; the wt tiles are
    write-once so no WAR edge can point at a removed load."""
    removed = 0
    for bb in nc.m.functions[0].blocks:
        insts = bb.instructions
        last_sig = None
        drop = set()
        pend = []
        for inst in insts:
            if inst.opcode == "Ldweights":
                ap = inst.ins[0]
                s = (ap.memref, ap.offset, str(ap.ap))
                if s == last_sig:
                    drop.add(inst.name)
                    pend.append(inst)
                else:
                    last_sig = s
            elif inst.opcode == "Matmult":
                for ldw in pend:
                    inst.merge_dependencies_from(ldw)
                pend.clear()
            elif inst.engine == mybir.EngineType.PE:
                # any other PE instruction invalidates the resident weights
                last_sig = None
        if drop:
            bb.instructions = [i for i in insts if i.name not in drop]
            removed += len(drop)
    return removed


def shard_inputs(x, w_mu, w_rho, b_mu, b_rho, w_eps, b_eps, drop_u):
    """Full inputs -> per-core in_maps (host-side layout/encoding prep)."""
    bf = ml_dtypes.bfloat16
    # mu enters host-prescaled by the 1/(1-p) dropout scale (folded constant)
    wmu_t = (np.asarray(w_mu, np.float32).T * SCALE).astype(bf)  # [IN, OUT]
    wrho_t = np.asarray(w_rho, np.float32).T.astype(bf)
    weps_t = np.asarray(w_eps, np.float32).T.astype(bf)
    # b[n] with n = s*128 + p  ->  [P, SC] table, column s
    bmu_r = np.asarray(b_mu, np.float32).reshape(SC, P).T * SCALE
    brho_r = np.asarray(b_rho, np.float32).reshape(SC, P).T.copy()
    beps_r = np.asarray(b_eps, np.float32).reshape(SC, P).T.copy()
    x = np.asarray(x, np.float32)
    drop_u = np.asarray(drop_u, np.float32)
    ball = np.ascontiguousarray(
        np.stack([bmu_r, brho_r, beps_r], axis=1), np.float32)  # [P, 3, SC]
    in_maps = []
    for c in range(N_CORES):
        sl = slice(c * BS, (c + 1) * BS)
        keep_t = (drop_u[sl] >= DROP).T                  # [OUT, BS] bool
        in_maps.append({
            "xt": x[sl].T.astype(bf),                    # [IN, BS] bf16
            "wmu": wmu_t, "wrho": wrho_t, "weps": weps_t,
            "ball": ball,
            "mk": np.where(keep_t, np.uint8(FP8_ONE),
                           np.uint8(0)),                 # fp8 bits in u8
        })
    return in_maps


def kernel(x, w_mu, w_rho, b_mu, b_rho, w_eps, b_eps, drop_u):
    nc = build_kernel()
    in_maps = shard_inputs(x, w_mu, w_rho, b_mu, b_rho, w_eps, b_eps, drop_u)
    res = run_bass_kernel_spmd(nc, in_maps, core_ids=list(range(N_CORES)))
    y = np.empty((B, OUT), np.float32)
    for c in range(N_CORES):
        yo = np.asarray(res.results[c]["yo"])            # [OUT, BS] bf16
        y[c * BS:(c + 1) * BS] = yo.astype(np.float32).T
    return y


# revision 52
# speedup vs baseline: 1.0025x; 1.0025x over previous
"""nn_BayesianLayer — reparameterized Bayesian linear layer + inverted dropout
on 8 TRN2 NeuronCores (data-parallel over the 65536-row batch).

reference:
  w = w_mu + softplus(w_rho) * w_eps            [512, 512]
  b = b_mu + softplus(b_rho) * b_eps            [512]
  y = (x @ w.T + b) * (drop_u >= 0.2) / 0.8     [65536, 512]

Sharding: x and drop_u split into 8 row-shards of 8192; weights replicated.
Each core runs the same single-core Bass/Tile graph (SPMD, no collectives);
outputs are gathered on the host.

This is a memory-regime problem (~51MB/core of fp32 traffic at a 358GB/s
per-core HBM ceiling), so the kernel is built around shrinking bytes moved
(the rel-err budget is 2e-2; measured end-to-end rel err ~2e-3):
 - x is fed host-transposed AND bf16 (xT [512, 8192], 8MB vs 16MB fp32).
 - drop_u enters as its information content: a host-encoded keep mask with
   fp8(e4m3) bit patterns {0.0, 1.0} in a uint8 tensor, transposed to
   [512, 8192] (4MB vs 16MB). On device it is bitcast to float8e4.
 - y leaves the device as bf16 (8MB) and is upcast to fp32 on the host.
 - the weight/rho/eps tensors are fed host-transposed bf16 (1.5MB).
Total ~21.5MB/core -> ~60us DMA roofline (vs ~142us for the fp32 layout).

The matmul is emitted output-transposed: yT[n, m] = sum_k w'T[k, n]*xT[k, m].
The stationary operand is a [128, 128] chunk of w'T (reused across 4 moving
passes - 4x fewer PE weight loads), and the bias lands on the PSUM partition
axis, so it can ride the per-partition "scalar" operand of a single fused
DVE/GPSIMD op or the ACT bias port -- no 5th bias matmul pass (TensorE stays
at its 4-pass compute floor, ~55us at 2.4GHz). Per 128x512 psum tile ONE
fused op applies bias+mask+downcast:
    yT_tile = (psum + b'[n]) * mask      (scalar_tensor_tensor, add/mult)
Tiles alternate between the DVE (direct from PSUM) and an ACT(bias via
Identity-activation bias port, PSUM->SBUF bf16) -> GPSIMD(mask multiply)
pipeline, because GPSIMD has no PSUM port and the DVE alone (1x mode from
PSUM + drains) would be the bottleneck.

Weight prologue (on device): w' = 1.25*(w_mu + softplus(w_rho)*w_eps).T in
bf16. softplus(rho) = ln1p(exp(rho)) for rho<0 (always true here); exp on
ACT, ln1p via a 4-term poly on DVE/GPSIMD (chunks split across both so the
serial chain that gates the first matmul stays short; ACT tables lack
Softplus/Ln). The 1/(1-p) dropout scale is folded into w', b'.

Main loop: 4 groups of 2048 batch rows; per group 2MB xs + 1MB mask slabs in,
2MB yT out, each split half/half across the SP and ACT HWDGE rings so both
descriptor rings stay busy at every instant.
"""

import numpy as np
import ml_dtypes

import concourse.bass as bass
import concourse.mybir as mybir
from concourse import bacc, tile
from concourse.bass import ts
from concourse.bass_utils import run_bass_kernel_spmd

AF = mybir.ActivationFunctionType
ALU = mybir.AluOpType

N_CORES = 8
B, IN, OUT = 65536, 512, 512
BS = B // N_CORES          # 8192 rows per core
P = 128
KC = IN // P               # 4 contraction chunks
SC = OUT // P              # 4 output (n) slices
GROUPS = 4                 # batch groups per core (default)
TW = 512                   # moving-tile width (psum free dim; max legal 512)
CW = 512                   # consumer-op chunk width
DROP = 0.2
SCALE = 1.0 / (1.0 - DROP)
FP8_ONE = 0x38             # float8e4 bit pattern of 1.0

# softplus(rho) on rho in [-3.7, -2.3] (rho = -3.0 + 0.1*randn; +-7 sigma)
# as a cubic Chebyshev fit in nested (u + b)*rho form -- 3 DVE ops, no Exp,
# no ACT on the prologue critical path. Max abs err 7.8e-5 (vs the ~4e-3
# bf16 rounding floor of this kernel). SP_C3 carries the folded 1/(1-p)
# dropout scale.
SP_B2 = 12.357892265882713
SP_B1 = 54.021235707244024
SP_B0 = 84.88210397568128
SP_C3 = 0.006898174577132781


def build_kernel(reps=None, dve8=6,
                 groups=(1024, 2048, 2048, 2048, 1024),
                 tw=TW, xbufs=3, obufs=2, out_split=True, order=2,
                 dma_only=False, pe_only=False, warmup=10):
    if isinstance(groups, int):
        groups = (BS // groups,) * groups
    assert sum(groups) == BS
    nc = bacc.Bacc(None, target_bir_lowering=False, debug=False)
    f32 = mybir.dt.float32
    bf16 = mybir.dt.bfloat16
    u8 = mybir.dt.uint8
    f8 = mybir.dt.float8e4

    xt = nc.declare_dram_parameter("xt", [IN, BS], bf16, isOutput=False)
    wmu = nc.declare_dram_parameter("wmu", [IN, OUT], bf16, isOutput=False)
    wrho = nc.declare_dram_parameter("wrho", [IN, OUT], bf16, isOutput=False)
    weps = nc.declare_dram_parameter("weps", [IN, OUT], bf16, isOutput=False)
    ball = nc.declare_dram_parameter("ball", [P, 3, SC], f32, isOutput=False)
    mk = nc.declare_dram_parameter("mk", [OUT, BS], u8, isOutput=False)
    yo = nc.declare_dram_parameter("yo", [OUT, BS], bf16, isOutput=True)

    xt_r = xt[:, :].rearrange("(k p) m -> p k m", p=P)    # [128, KC, BS]
    wmu_r = wmu[:, :].rearrange("(k p) n -> p k n", p=P)  # [128, KC, OUT]
    wrho_r = wrho[:, :].rearrange("(k p) n -> p k n", p=P)
    weps_r = weps[:, :].rearrange("(k p) n -> p k n", p=P)
    mk_r = mk[:, :].rearrange("(s p) m -> p s m", p=P)    # [128, SC, BS]
    yo_r = yo[:, :].rearrange("(s p) m -> p s m", p=P)

    with tile.TileContext(nc) as tc:
        with (
            tc.tile_pool(name="wt", bufs=1) as wt_pool,
            tc.tile_pool(name="prol", bufs=2) as prol_pool,
            tc.tile_pool(name="bias", bufs=1) as bias_pool,
            tc.tile_pool(name="xs", bufs=xbufs) as x_pool,
            tc.tile_pool(name="mks", bufs=xbufs) as mk_pool,
            tc.tile_pool(name="outs", bufs=obufs) as out_pool,
            tc.tile_pool(name="tmp", bufs=6) as tmp_pool,
            tc.tile_pool(name="ps", bufs=8 * 512 // tw,
                         space="PSUM") as psum_pool,
        ):
            # ---- weight prologue: w'T = 1.25*(mu + ln1p(exp(rho))*eps).T,
            # per k-chunk in bf16: exp on ACT; a = (t*-0.5 + 1) on DVE; then
            # b = t*eps, c = a*b, w = c+mu as tensor-tensor ops alternating
            # DVE/GPSIMD per chunk; cast+scale on ACT. ----
            # mu arrives host-prescaled by 1.25 (the dropout scale); the
            # same 1.25 is folded into SP_C3, so w' = mu' + sp'(rho)*eps
            # needs no extra scale/cast: the final add writes the bf16 wt
            # tile directly. Poly on DVE; tails alternate DVE/GPSIMD.
            # DMA order is tuned for the serial start: rho first (gates the
            # poly), then group 0's input slabs, then eps/mu/bias.
            from collections import Counter
            size_count = Counter(groups)

            def prefetch_group(m0, gb):
                h = gb // 2
                bfs = min(size_count[gb], xbufs)
                xs = x_pool.tile([P, KC, gb], bf16, tag=f"xs{gb}",
                                 name=f"xs{gb}", bufs=bfs)
                nc.sync.dma_start(out=xs[:, :, :h],
                                  in_=xt_r[:, :, m0:m0 + h])
                nc.scalar.dma_start(out=xs[:, :, h:],
                                    in_=xt_r[:, :, m0 + h:m0 + gb])
                mks = mk_pool.tile([P, SC, gb], u8, tag=f"mks{gb}",
                                   name=f"mks{gb}", bufs=bfs)
                nc.sync.dma_start(out=mks[:, :, :h],
                                  in_=mk_r[:, :, m0:m0 + h])
                nc.scalar.dma_start(out=mks[:, :, h:],
                                    in_=mk_r[:, :, m0 + h:m0 + gb])
                return xs, mks

            rho_a = prol_pool.tile([P, KC, OUT], bf16, tag="rho", bufs=1)
            mu_a = prol_pool.tile([P, KC, OUT], bf16, tag="mu", bufs=1)
            eps_a = prol_pool.tile([P, KC, OUT], bf16, tag="eps", bufs=1)
            # ---- PE warm-up: dummy matmuls on scratch data fill the PE's
            # otherwise-idle prologue window so the DVFS p-state is fully
            # ramped when the first real matmul issues. ----
            if warmup and not dma_only:
                wu_s = bias_pool.tile([P, OUT], bf16, tag="wu_s")
                nc.vector.memset(wu_s[:], 1.0)
                wu_p = psum_pool.tile([P, tw], f32, tag="ps", name="wu_p")
                for i in range(warmup):
                    nc.tensor.matmul(wu_p[:], wu_s[:, :P], wu_s[:, :tw],
                                     start=(i == 0), stop=(i == warmup - 1))

            g0_tiles = None
            if order == 0:
                nc.sync.dma_start(out=rho_a[:], in_=wrho_r[:, :])
                if reps is None:
                    g0_tiles = prefetch_group(0, groups[0])
                nc.scalar.dma_start(out=eps_a[:], in_=weps_r[:, :])
                nc.scalar.dma_start(out=mu_a[:], in_=wmu_r[:, :])
            elif order == 1:
                nc.scalar.dma_start(out=rho_a[:], in_=wrho_r[:, :])
                nc.sync.dma_start(out=eps_a[:], in_=weps_r[:, :])
                nc.sync.dma_start(out=mu_a[:], in_=wmu_r[:, :])
                if reps is None:
                    g0_tiles = prefetch_group(0, groups[0])
            else:
                nc.sync.dma_start(out=rho_a[:], in_=wrho_r[:, :])
                nc.scalar.dma_start(out=eps_a[:], in_=weps_r[:, :])
                nc.sync.dma_start(out=mu_a[:], in_=wmu_r[:, :])
                if reps is None:
                    g0_tiles = prefetch_group(0, groups[0])
            wt = []
            for k in range(KC):
                rho_k = rho_a[:, k]
                sp = prol_pool.tile([P, OUT], f32, tag="sp")
                nc.vector.scalar_tensor_tensor(
                    sp[:], rho_k, SP_B2, rho_k, ALU.add, ALU.mult)
                nc.vector.scalar_tensor_tensor(
                    sp[:], sp[:], SP_B1, rho_k, ALU.add, ALU.mult)
                nc.vector.tensor_scalar(sp[:], sp[:], SP_B0, SP_C3 * SCALE,
                                        ALU.add, ALU.mult)
                eng = nc.gpsimd if k % 2 == 0 else nc.vector
                eng.tensor_mul(sp[:], sp[:], eps_a[:, k])
                wtk = wt_pool.tile([P, OUT], bf16, tag=f"wt{k}")
                eng.tensor_add(wtk[:], sp[:], mu_a[:, k])
                wt.append(wtk)

            # ---- bias prologue: b' as a [128, SC] per-partition table;
            # column s is the bias vector for output slice s. The three
            # bias tensors arrive packed in one [P, 3, SC] DMA. ----
            ball_t = bias_pool.tile([P, 3, SC], f32, tag="ball")
            nc.scalar.dma_start(out=ball_t[:], in_=ball[:, :, :])
            bmu_t, brho_t, beps_t = ball_t[:, 0], ball_t[:, 1], ball_t[:, 2]
            spb = bias_pool.tile([P, SC], f32, tag="spb")
            nc.vector.scalar_tensor_tensor(
                spb[:], brho_t, SP_B2, brho_t, ALU.add, ALU.mult)
            nc.vector.scalar_tensor_tensor(
                spb[:], spb[:], SP_B1, brho_t, ALU.add, ALU.mult)
            nc.vector.tensor_scalar(spb[:], spb[:], SP_B0, SP_C3 * SCALE,
                                    ALU.add, ALU.mult)
            nc.vector.tensor_mul(spb[:], spb[:], beps_t)
            b_vec = bias_pool.tile([P, SC], f32, tag="bvec")
            nc.vector.tensor_add(b_vec[:], spb[:], bmu_t)

            # ---- main loop (uneven groups: small first group so output
            # stores start early, small last group so the tail drains) ----
            c_counter = [0]
            N_CHUNKS = SC * BS // CW     # 64 consumer chunks total

            def emit_group(m0, gb, pre=None):
                h = gb // 2
                if pe_only:
                    bfs = min(size_count[gb], xbufs)
                    xs = x_pool.tile([P, KC, gb], bf16, tag=f"xs{gb}",
                                     name=f"xs{gb}", bufs=bfs)
                    nc.sync.dma_start(out=xs[:, :, :h],
                                      in_=xt_r[:, :, m0:m0 + h])
                    nc.scalar.dma_start(out=xs[:, :, h:],
                                        in_=xt_r[:, :, m0 + h:m0 + gb])
                    for s in range(SC):
                        pss = [psum_pool.tile([P, tw], f32, tag="ps",
                                              name=f"pso{mb}")
                               for mb in range(gb // tw)]
                        for k in range(KC):
                            for mb in range(gb // tw):
                                nc.tensor.matmul(
                                    pss[mb][:], wt[k][:, ts(s, P)],
                                    xs[:, k, mb * tw:(mb + 1) * tw],
                                    start=(k == 0), stop=(k == KC - 1))
                    return
                xs, mks = pre if pre is not None else prefetch_group(m0, gb)
                outs = out_pool.tile([P, SC, gb], bf16, tag=f"outs{gb}",
                                     name=f"outs{gb}",
                                     bufs=min(size_count[gb], obufs))
                mbn = gb // tw if not dma_only else 0
                for s in range(SC):
                    # k-outer / mb-inner: consecutive matmuls share the
                    # stationary wt[k][:, s], so dedup_ldweights() can strip
                    # the redundant PE weight reloads (mbn-1 of every mbn).
                    pss = [psum_pool.tile([P, tw], f32, tag="ps",
                                          name=f"ps{mb}")
                           for mb in range(mbn)]
                    for k in range(KC):
                        for mb in range(mbn):
                            nc.tensor.matmul(
                                pss[mb][:], wt[k][:, ts(s, P)],
                                xs[:, k, mb * tw:(mb + 1) * tw],
                                start=(k == 0), stop=(k == KC - 1))
                    for mb in range(mbn):
                        ps = pss[mb]
                        # consumers in CW-wide chunks split across engines:
                        # A: one fused DVE op straight from PSUM;
                        # B: ACT bias-add (psum->sbuf bf16) + GPSIMD mask.
                        for q in range(tw // CW):
                            c_idx = c_counter[0]
                            c_counter[0] += 1
                            c0 = mb * tw + q * CW
                            o_sl = outs[:, s, c0:c0 + CW]
                            m_sl = mks[:, s, c0:c0 + CW].bitcast(f8)
                            p_sl = ps[:, ts(q, CW)]
                            if c_idx % 8 < dve8 or c_idx >= N_CHUNKS - 4:
                                nc.vector.scalar_tensor_tensor(
                                    o_sl, p_sl, b_vec[:, ts(s, 1)], m_sl,
                                    ALU.add, ALU.mult)
                            else:
                                tmp = tmp_pool.tile([P, CW], bf16, tag="tmp")
                                nc.scalar.activation(
                                    tmp[:], p_sl, AF.Identity,
                                    bias=b_vec[:, ts(s, 1)], scale=1.0)
                                nc.gpsimd.tensor_mul(o_sl, tmp[:], m_sl)
                    if out_split:
                        # store each n-slice as soon as its consumers finish
                        eng = nc.scalar if s % 2 == 0 else nc.sync
                        src = outs[:, s] if not dma_only else xs[:, s]
                        eng.dma_start(out=yo_r[:, s, m0:m0 + gb], in_=src)
                if not out_split:
                    nc.scalar.dma_start(out=yo_r[:, :, m0:m0 + h],
                                        in_=outs[:, :, :h])
                    nc.sync.dma_start(out=yo_r[:, :, m0 + h:m0 + gb],
                                      in_=outs[:, :, h:])

            def emit_all(first_pre):
                m0 = 0
                for gi, gb in enumerate(groups):
                    emit_group(m0, gb, pre=first_pre if gi == 0 else None)
                    m0 += gb

            if reps is None:
                emit_all(g0_tiles)
            else:
                with tc.For_i(0, reps) as _:
                    emit_all(None)

    dedup_ldweights(nc)
    nc.finalize()
    return nc


def dedup_ldweights(nc):
    """Remove PE weight reloads that re-load the stationary already resident
    in the PE array. The tile scheduler emits one InstLdweights per matmul;
    a Matmult NOT preceded by its own Ldweights uses the resident stationary
    (HW-verified: exact results, transitions included). Each removed load
    saves ~128 PE cycles. Dependency edges of a removed Ldweights are merged
    into the next Matmult so tile-level sync is preserved; the wt tiles are
    write-once so no WAR edge can point at a removed load."""
    removed = 0
    for bb in nc.m.functions[0].blocks:
        insts = bb.instructions
        last_sig = None
        drop = set()
        pend = []
        for inst in insts:
            if inst.opcode == "Ldweights":
                ap = inst.ins[0]
                s = (ap.memref, ap.offset, str(ap.ap))
                if s == last_sig:
                    drop.add(inst.name)
                    pend.append(inst)
                else:
                    last_sig = s
            elif inst.opcode == "Matmult":
                for ldw in pend:
                    inst.merge_dependencies_from(ldw)
                pend.clear()
            elif inst.engine == mybir.EngineType.PE:
                # any other PE instruction invalidates the resident weights
                last_sig = None
        if drop:
            bb.instructions = [i for i in insts if i.name not in drop]
            removed += len(drop)
    return removed


def shard_inputs(x, w_mu, w_rho, b_mu, b_rho, w_eps, b_eps, drop_u):
    """Full inputs -> per-core in_maps (host-side layout/encoding prep)."""
    bf = ml_dtypes.bfloat16
    # mu enters host-prescaled by the 1/(1-p) dropout scale (folded constant)
    wmu_t = (np.asarray(w_mu, np.float32).T * SCALE).astype(bf)  # [IN, OUT]
    wrho_t = np.asarray(w_rho, np.float32).T.astype(bf)
    weps_t = np.asarray(w_eps, np.float32).T.astype(bf)
    # b[n] with n = s*128 + p  ->  [P, SC] table, column s
    bmu_r = np.asarray(b_mu, np.float32).reshape(SC, P).T * SCALE
    brho_r = np.asarray(b_rho, np.float32).reshape(SC, P).T.copy()
    beps_r = np.asarray(b_eps, np.float32).reshape(SC, P).T.copy()
    x = np.asarray(x, np.float32)
    drop_u = np.asarray(drop_u, np.float32)
    ball = np.ascontiguousarray(
        np.stack([bmu_r, brho_r, beps_r], axis=1), np.float32)  # [P, 3, SC]
    in_maps = []
    for c in range(N_CORES):
        sl = slice(c * BS, (c + 1) * BS)
        keep_t = (drop_u[sl] >= DROP).T                  # [OUT, BS] bool
        in_maps.append({
            "xt": x[sl].T.astype(bf),                    # [IN, BS] bf16
            "wmu": wmu_t, "wrho": wrho_t, "weps": weps_t,
            "ball": ball,
            "mk": np.where(keep_t, np.uint8(FP8_ONE),
                           np.uint8(0)),                 # fp8 bits in u8
        })
    return in_maps


def kernel(x, w_mu, w_rho, b_mu, b_rho, w_eps, b_eps, drop_u):
    nc = build_kernel()
    in_maps = shard_inputs(x, w_mu, w_rho, b_mu, b_rho, w_eps, b_eps, drop_u)
    res = run_bass_kernel_spmd(nc, in_maps, core_ids=list(range(N_CORES)))
    y = np.empty((B, OUT), np.float32)
    for c in range(N_CORES):
        yo = np.asarray(res.results[c]["yo"])            # [OUT, BS] bf16
        y[c * BS:(c + 1) * BS] = yo.astype(np.float32).T
    return y
